# revision 15
# baseline (speedup 1.0000x reference)
"""Trainium2 Bass kernel for nn_MemoryGame (scatter_memory).

Strategy (8 NeuronCores, tensor-parallel over M's columns):
  - Tiny MLPs / outer products / readout run on host (microseconds of work).
  - The heavy part (50-iteration Hopfield loop over M [9216,9216] plus the
    rank-1 Hebbian update of M) runs on 8 cores.
  - M's columns are sharded interleaved: core c owns columns j with
    (j % 128)//16 == c, stored locally in (p', f) order where the global
    column is j = f*128 + 16c + p'.  With this order, the per-iteration
    AllGather of the 8 local h-slices concatenates into exactly the
    partition-major [128, 72] layout the TensorEngine needs for the next
    matvec -- every DMA stays contiguous.
  - M is cast to fp16 and kept resident in SBUF (21.2MB/core): HBM traffic is
    one 42.5MB read (load) and one 42.5MB write (M_out) per core, total.
  - h state stays f32 (it decays to ~1e-5, below fp16-normal range); the
    fp16 rounding only enters the matvec input, where its effect on the
    update factor (kappa + h@M) is suppressed by kappa.
  - The AllGather transports the fp16 matvec copy of h; dummy matmuls keep
    the PE's HAM clock-gate warm across the gather tail.
"""

import numpy as np

P = 9216
DIM_X = 96
DIM_G = 96
NUM_CLASS = 1000
N_ITER = 50
KAPPA, LAMDA, YITA = 0.8, 0.9, 0.1
LEAKY_SLOPE = 0.01

NCORES = 8
LOCAL = P // NCORES          # 1152 columns per core
F = P // 128                 # 72 k-chunks / free size of pm layout
PF = 128 // NCORES           # 16 partitions per core in the interleave
NSPLITS = [(0, 512), (512, 512), (1024, 128)]  # LOCAL split into <=512 psum slices

_BUILD_CACHE = {}


def _build_nc(n_iter=N_ITER, use_cc=True, warm_dummies=0, col_tile=0):
    import concourse.bacc as bacc
    import concourse.mybir as mybir
    import concourse.tile as tile

    f32 = mybir.dt.float32
    f16 = mybir.dt.float16
    Alu = mybir.AluOpType

    nc = bacc.Bacc("TRN2", target_bir_lowering=False, num_devices=NCORES)

    m_in = nc.dram_tensor("m_in", [P, LOCAL], f32, kind="ExternalInput")
    h0pm = nc.dram_tensor("h0pm", [128, F], f16, kind="ExternalInput")
    h0loc = nc.dram_tensor("h0loc", [1, LOCAL], f32, kind="ExternalInput")
    ppm = nc.dram_tensor("ppm", [128, F], f32, kind="ExternalInput")
    ploc = nc.dram_tensor("ploc", [1, LOCAL], f32, kind="ExternalInput")
    pout = nc.dram_tensor("pout", [1, LOCAL], f32, kind="ExternalOutput")
    mout = nc.dram_tensor("mout", [P, LOCAL], f32, kind="ExternalOutput")

    rg = [list(range(NCORES))]

    with tile.TileContext(nc) as tc:
        with (
            tc.tile_pool(name="mpool", bufs=1) as mpool,
            tc.tile_pool(name="stage", bufs=2) as stpool,
            tc.tile_pool(name="small", bufs=1) as sm,
            tc.tile_pool(name="rot", bufs=2) as rot,
            tc.tile_pool(name="psum", bufs=1, space="PSUM") as ps,
            tc.tile_pool(name="dram", bufs=max(n_iter, 2), space="DRAM") as dr,
        ):
            # ---- Phase A: load M shard, cast f32 -> fp16 into SBUF ----
            M16s = []
            for t in range(F):
                mt = mpool.tile([128, LOCAL], f16, tag=f"m{t}", name=f"m{t}")
                M16s.append(mt)
                stg = stpool.tile([128, LOCAL], f32, tag="stage", name="stg")
                nc.sync.dma_start(stg[:], m_in[t * 128:(t + 1) * 128, :])
                nc.vector.tensor_copy(mt[:], stg[:])

            # ---- persistent small tiles ----
            p_pm = sm.tile([128, F], f32, tag="p_pm")
            nc.sync.dma_start(p_pm[:], ppm[:])
            u = sm.tile([1, LOCAL], f32, tag="u")
            v = sm.tile([1, LOCAL], f32, tag="v")
            h16d = sm.tile([128, 1], f16, tag="h16d")  # constant dummy weights
            nc.vector.memset(h16d[:], 0.0)

            h_pm = rot.tile([128, F], f16, tag="hpm", name="hpm0")
            nc.sync.dma_start(h_pm[:], h0pm[:])
            h_loc = rot.tile([1, LOCAL], f32, tag="hloc", name="hloc0")
            nc.sync.dma_start(h_loc[:], h0loc[:])

            if col_tile:
                acc = ps.tile([128, 512], f32, tag="acc", name="acc")
                csz = LOCAL // col_tile
            else:
                acc = ps.tile([1, 1536], f32, tag="acc", name="acc")
            scr = ps.tile([1, 512], f32, tag="scr", name="scr")
            dbg = dr.tile([1, 1], f32, tag="dbg", bufs=1, name="dbg")

            def dummy_mm(dep_ap=None):
                # keep the PE HAM-warm; reads a constant weight + M chunk 0
                lhsT = dep_ap if dep_ap is not None else h16d[:]
                nc.tensor.matmul(scr[:], lhsT, M16s[0][:, 0:512],
                                 start=True, stop=True, skip_group_check=True)

            # ---- Phase B: Hopfield attractor loop ----
            for it in range(n_iter):
                if col_tile:
                    for t in range(F):
                        lhsT = h_pm[:, t:t + 1]
                        for g in range(col_tile):
                            nc.tensor.matmul(
                                acc[32 * g:32 * g + 1, 0:csz],
                                lhsT,
                                M16s[t][:, g * csz:(g + 1) * csz],
                                start=(t == 0), stop=(t == F - 1),
                                tile_position=(0, 32 * g),
                            )
                else:
                    for t in range(F):
                        lhsT = h_pm[:, t:t + 1]
                        for off, sz in NSPLITS:
                            nc.tensor.matmul(
                                acc[:, off:off + sz],
                                lhsT,
                                M16s[t][:, off:off + sz],
                                start=(t == 0), stop=(t == F - 1),
                            )
                # u = (hM + kappa) * h
                if col_tile:
                    for g in range(col_tile):
                        nc.vector.scalar_tensor_tensor(
                            u[:, g * csz:(g + 1) * csz],
                            acc[32 * g:32 * g + 1, 0:csz], KAPPA,
                            h_loc[:, g * csz:(g + 1) * csz],
                            Alu.add, Alu.mult,
                        )
                else:
                    nc.vector.scalar_tensor_tensor(
                        u[:], acc[:, 0:LOCAL], KAPPA, h_loc[:],
                        Alu.add, Alu.mult,
                    )
                if warm_dummies:
                    dummy_mm()
                # leaky relu: v = max(u*0.01, u)
                nc.vector.scalar_tensor_tensor(
                    v[:], u[:], LEAKY_SLOPE, u[:], Alu.mult, Alu.max
                )
                if warm_dummies:
                    dummy_mm()
                # clamp to [-1, 1]
                h_loc_new = rot.tile([1, LOCAL], f32, tag="hloc", name="hloc")
                nc.vector.tensor_scalar(
                    h_loc_new[:], v[:], 1.0, -1.0, Alu.min, Alu.max
                )
                # fp16 transport copy
                h16loc = rot.tile([1, LOCAL], f16, tag="h16loc", name="h16loc",
                                  bufs=1)
                nc.vector.tensor_copy(h16loc[:], h_loc_new[:])
                if warm_dummies:
                    for _ in range(warm_dummies):
                        dummy_mm()

                # all-gather the 8 local slices into pm order (fp16)
                bin_ = dr.tile([1, LOCAL], f16, tag="bin", name="bin")
                bout = dr.tile([NCORES, LOCAL], f16, tag="bout", name="bout")
                nc.sync.dma_start(bin_[:], h16loc[:])
                if use_cc:
                    nc.gpsimd.collective_compute(
                        "AllGather", Alu.bypass, replica_groups=rg,
                        ins=[bin_[:].opt()], outs=[bout[:].opt()],
                    )
                else:  # debug: fake gather (wrong data, same dataflow)
                    for r in range(NCORES):
                        nc.sync.dma_start(bout[r:r + 1, :], bin_[:])
                h_pm = rot.tile([128, F], f16, tag="hpm", name="hpm")
                nc.sync.dma_start(
                    h_pm[:], bout[:].rearrange("a (b c) -> (a b) c", b=PF)
                )
                h_loc = h_loc_new

            # keep the dummy psum live so DCE can't drop the warm-up matmuls
            if warm_dummies:
                nc.vector.tensor_copy(u[:, 0:1], scr[0:1, 0:1])
                nc.sync.dma_start(dbg[:], u[:, 0:1])

            # ---- outputs: p_ slice ----
            nc.sync.dma_start(pout[:], h_loc[:])

            # ---- Phase D: M_out = 0.9*M + 0.1*(p+p_)(p-p_)^T (column shard) ----
            a_pm = sm.tile([128, F], f32, tag="apm")
            nc.vector.tensor_add(a_pm[:], p_pm[:], h_pm[:])
            # bdiff = p_loc - h_loc, staged through the (now dead) v tile
            nc.sync.dma_start(v[:], ploc[:])
            nc.vector.tensor_sub(v[:], v[:], h_loc[:])
            # broadcast 0.1*bdiff across 128 partitions via K=1 outer product
            oneY = sm.tile([1, 128], f32, tag="oneY")
            nc.vector.memset(oneY[:], YITA)
            B01 = sm.tile([128, LOCAL], f32, tag="B01")
            for (off, sz), tg in zip(NSPLITS, ["bb0", "bb1", "bb2"]):
                bb = ps.tile([128, sz], f32, tag=tg, name=tg)
                nc.tensor.matmul(
                    bb[:], oneY[:], v[:, off:off + sz], start=True, stop=True
                )
                nc.vector.tensor_copy(B01[:, off:off + sz], bb[:])
            for t in range(F):
                w = rot.tile([128, LOCAL], f32, tag="w", name="w")
                ot = stpool.tile([128, LOCAL], f32, tag="stage", name="ot")
                nc.gpsimd.tensor_scalar_mul(w[:], B01[:], a_pm[:, t:t + 1])
                nc.vector.scalar_tensor_tensor(
                    ot[:], M16s[t][:], LAMDA, w[:], Alu.mult, Alu.add,
                )
                nc.sync.dma_start(mout[t * 128:(t + 1) * 128, :], ot[:])

    nc.compile()
    return nc


def _get_nc():
    if "nc" not in _BUILD_CACHE:
        _BUILD_CACHE["nc"] = _build_nc()
    return _BUILD_CACHE["nc"]


def _col_idx(c):
    """Global column indices owned by core c, in local (p'-major) order."""
    J = np.arange(P, dtype=np.int64).reshape(F, 128)  # J[f, p] = f*128 + p
    return J[:, PF * c: PF * (c + 1)].T.reshape(-1)   # (p', f) -> f*128+16c+p'


def _pm(vec):
    """[P] vector -> partition-major [128, F]: out[p, f] = vec[f*128 + p]."""
    return np.ascontiguousarray(vec.reshape(F, 128).T)


def kernel(x, g, M, W1x, b1x, W2x, b2x, W1g, b1g, W2g, b2g, Wp, bp):
    from concourse.bass_utils import run_bass_kernel_spmd

    f32 = np.float32
    x = np.asarray(x, f32); g = np.asarray(g, f32); M = np.asarray(M, f32)

    # ---- host: tiny MLPs, p, query, h0 ----
    def mlp2(inp, W1, b1, W2, b2):
        h = inp @ np.asarray(W1, f32) + np.asarray(b1, f32)
        return np.maximum(h, 0.0) @ np.asarray(W2, f32) + np.asarray(b2, f32)

    x_ = mlp2(x, W1x, b1x, W2x, b2x)[0]   # [96]
    g_ = mlp2(g, W1g, b1g, W2g, b2g)[0]   # [96]
    p = np.outer(x_, g_).reshape(-1).astype(f32)       # [9216]
    query = np.tile(g_, DIM_X).astype(f32)             # [9216]
    h0 = np.clip(np.maximum(query, LEAKY_SLOPE * query), -1.0, 1.0).astype(f32)

    M0 = M.reshape(P, P)
    arr = M0.reshape(P, F, 128)
    h0_pm = _pm(h0)
    p_pm = _pm(p)

    in_maps = []
    for c in range(NCORES):
        shard = np.ascontiguousarray(
            arr[:, :, PF * c: PF * (c + 1)].transpose(0, 2, 1).reshape(P, LOCAL)
        )
        h0_loc = np.ascontiguousarray(h0_pm[PF * c: PF * (c + 1), :]).reshape(1, LOCAL)
        p_loc = np.ascontiguousarray(p_pm[PF * c: PF * (c + 1), :]).reshape(1, LOCAL)
        in_maps.append({
            "m_in": shard,
            "h0pm": h0_pm.astype(np.float16),
            "h0loc": h0_loc,
            "ppm": p_pm,
            "ploc": p_loc,
        })

    nc = _get_nc()
    res = run_bass_kernel_spmd(nc, in_maps, core_ids=list(range(NCORES)))
    results = res.results

    # ---- host: reassemble ----
    p_pm_full = np.empty((128, F), f32)
    R = np.empty((P, F, 128), f32)
    for c in range(NCORES):
        p_pm_full[PF * c: PF * (c + 1), :] = results[c]["pout"].reshape(PF, F)
        R[:, :, PF * c: PF * (c + 1)] = (
            results[c]["mout"].reshape(P, PF, F).transpose(0, 2, 1)
        )
    p_ = np.ascontiguousarray(p_pm_full.T).reshape(1, P)
    M_out = R.reshape(1, P, P)

    x_out = p_.reshape(DIM_X, DIM_G).sum(axis=1)
    x_inf = (x_out @ np.asarray(Wp, f32) + np.asarray(bp, f32)).astype(f32)

    return x_inf, p_, M_out


# revision 17
# speedup vs baseline: 2.1145x; 2.1145x over previous
"""Trainium2 Bass kernel for nn_MemoryGame (scatter_memory).

Strategy (8 NeuronCores, tensor-parallel over M's columns):
  - Tiny MLPs / outer products / readout run on host (microseconds of work).
  - The heavy part (50-iteration Hopfield loop over M [9216,9216] plus the
    rank-1 Hebbian update of M) runs on 8 cores.
  - M's columns are sharded interleaved: core c owns columns j with
    (j % 128)//16 == c, stored locally in (p', f) order where the global
    column is j = f*128 + 16c + p'.  With this order, the per-iteration
    AllGather of the 8 local h-slices concatenates into exactly the
    partition-major [128, 72] layout the TensorEngine needs for the next
    matvec -- every DMA stays contiguous.
  - M is cast to fp16 and kept resident in SBUF (21.2MB/core): HBM traffic is
    one 42.5MB read (load) and one 42.5MB write (M_out) per core, total.
  - h state stays f32 (it decays to ~1e-5, below fp16-normal range); the
    fp16 rounding only enters the matvec input, where its effect on the
    update factor (kappa + h@M) is suppressed by kappa.
  - The AllGather transports the fp16 matvec copy of h; dummy matmuls keep
    the PE's HAM clock-gate warm across the gather tail.
"""

import numpy as np

P = 9216
DIM_X = 96
DIM_G = 96
NUM_CLASS = 1000
N_ITER = 50
KAPPA, LAMDA, YITA = 0.8, 0.9, 0.1
LEAKY_SLOPE = 0.01

NCORES = 8
LOCAL = P // NCORES          # 1152 columns per core
F = P // 128                 # 72 k-chunks / free size of pm layout
PF = 128 // NCORES           # 16 partitions per core in the interleave
NSPLITS = [(0, 512), (512, 512), (1024, 128)]  # LOCAL split into <=512 psum slices

_BUILD_CACHE = {}


def _build_nc(n_iter=N_ITER, use_cc=True, warm_dummies=0, col_tile=0):
    import concourse.bacc as bacc
    import concourse.mybir as mybir
    import concourse.tile as tile

    f32 = mybir.dt.float32
    f16 = mybir.dt.float16
    Alu = mybir.AluOpType

    nc = bacc.Bacc("TRN2", target_bir_lowering=False, num_devices=NCORES)

    m_in = nc.dram_tensor("m_in", [P, LOCAL], f32, kind="ExternalInput")
    h0pm = nc.dram_tensor("h0pm", [128, F], f16, kind="ExternalInput")
    h0loc = nc.dram_tensor("h0loc", [1, LOCAL], f32, kind="ExternalInput")
    ppm = nc.dram_tensor("ppm", [128, F], f32, kind="ExternalInput")
    ploc = nc.dram_tensor("ploc", [1, LOCAL], f32, kind="ExternalInput")
    pout = nc.dram_tensor("pout", [1, LOCAL], f32, kind="ExternalOutput")
    mout = nc.dram_tensor("mout", [P, LOCAL], f32, kind="ExternalOutput")

    rg = [list(range(NCORES))]

    with tile.TileContext(nc) as tc:
        with (
            tc.tile_pool(name="mpool", bufs=1) as mpool,
            tc.tile_pool(name="stage", bufs=2) as stpool,
            tc.tile_pool(name="small", bufs=1) as sm,
            tc.tile_pool(name="rot", bufs=2) as rot,
            tc.tile_pool(name="psum", bufs=1, space="PSUM") as ps,
            tc.tile_pool(name="dram", bufs=max(n_iter, 2), space="DRAM") as dr,
        ):
            # ---- Phase A: load M shard, cast f32 -> fp16 into SBUF ----
            M16s = []
            for t in range(F):
                mt = mpool.tile([128, LOCAL], f16, tag=f"m{t}", name=f"m{t}")
                M16s.append(mt)
                stg = stpool.tile([128, LOCAL], f32, tag="stage", name="stg")
                nc.sync.dma_start(stg[:], m_in[t * 128:(t + 1) * 128, :])
                nc.vector.tensor_copy(mt[:], stg[:])

            # ---- persistent small tiles ----
            p_pm = sm.tile([128, F], f32, tag="p_pm")
            nc.sync.dma_start(p_pm[:], ppm[:])
            u = sm.tile([1, LOCAL], f32, tag="u")
            v = sm.tile([1, LOCAL], f32, tag="v")
            h16d = sm.tile([128, 1], f16, tag="h16d")  # constant dummy weights
            nc.vector.memset(h16d[:], 0.0)

            h_pm = rot.tile([128, F], f16, tag="hpm", name="hpm0")
            nc.sync.dma_start(h_pm[:], h0pm[:])
            h_loc = rot.tile([1, LOCAL], f32, tag="hloc", name="hloc0")
            nc.sync.dma_start(h_loc[:], h0loc[:])

            if col_tile:
                acc = ps.tile([128, 512], f32, tag="acc", name="acc")
                csz = LOCAL // col_tile
            else:
                acc = ps.tile([1, 1536], f32, tag="acc", name="acc")
            scr = ps.tile([1, 512], f32, tag="scr", name="scr")
            dbg = dr.tile([1, 1], f32, tag="dbg", bufs=1, name="dbg")

            def dummy_mm(dep_ap=None):
                # keep the PE HAM-warm; reads a constant weight + M chunk 0
                lhsT = dep_ap if dep_ap is not None else h16d[:]
                nc.tensor.matmul(scr[:], lhsT, M16s[0][:, 0:512],
                                 start=True, stop=True, skip_group_check=True)

            # ---- Phase B: Hopfield attractor loop ----
            for it in range(n_iter):
                if col_tile:
                    for t in range(F):
                        lhsT = h_pm[:, t:t + 1]
                        for g in range(col_tile):
                            nc.tensor.matmul(
                                acc[32 * g:32 * g + 1, 0:csz],
                                lhsT,
                                M16s[t][:, g * csz:(g + 1) * csz],
                                start=(t == 0), stop=(t == F - 1),
                                tile_position=(0, 32 * g),
                            )
                else:
                    for t in range(F):
                        lhsT = h_pm[:, t:t + 1]
                        for off, sz in NSPLITS:
                            nc.tensor.matmul(
                                acc[:, off:off + sz],
                                lhsT,
                                M16s[t][:, off:off + sz],
                                start=(t == 0), stop=(t == F - 1),
                            )
                # u = (hM + kappa) * h
                if col_tile:
                    for g in range(col_tile):
                        nc.vector.scalar_tensor_tensor(
                            u[:, g * csz:(g + 1) * csz],
                            acc[32 * g:32 * g + 1, 0:csz], KAPPA,
                            h_loc[:, g * csz:(g + 1) * csz],
                            Alu.add, Alu.mult,
                        )
                else:
                    nc.vector.scalar_tensor_tensor(
                        u[:], acc[:, 0:LOCAL], KAPPA, h_loc[:],
                        Alu.add, Alu.mult,
                    )
                if warm_dummies:
                    dummy_mm()
                # leaky relu: v = max(u*0.01, u)
                nc.vector.scalar_tensor_tensor(
                    v[:], u[:], LEAKY_SLOPE, u[:], Alu.mult, Alu.max
                )
                if warm_dummies:
                    dummy_mm()
                # clamp to [-1, 1]
                h_loc_new = rot.tile([1, LOCAL], f32, tag="hloc", name="hloc")
                nc.vector.tensor_scalar(
                    h_loc_new[:], v[:], 1.0, -1.0, Alu.min, Alu.max
                )
                # fp16 transport copy
                h16loc = rot.tile([1, LOCAL], f16, tag="h16loc", name="h16loc",
                                  bufs=1)
                nc.vector.tensor_copy(h16loc[:], h_loc_new[:])
                if warm_dummies:
                    for _ in range(warm_dummies):
                        dummy_mm()

                # all-gather the 8 local slices into pm order (fp16)
                bin_ = dr.tile([1, LOCAL], f16, tag="bin", name="bin")
                bout = dr.tile([NCORES, LOCAL], f16, tag="bout", name="bout")
                nc.sync.dma_start(bin_[:], h16loc[:])
                if use_cc:
                    nc.gpsimd.collective_compute(
                        "AllGather", Alu.bypass, replica_groups=rg,
                        ins=[bin_[:].opt()], outs=[bout[:].opt()],
                    )
                else:  # debug: fake gather (wrong data, same dataflow)
                    for r in range(NCORES):
                        nc.sync.dma_start(bout[r:r + 1, :], bin_[:])
                h_pm = rot.tile([128, F], f16, tag="hpm", name="hpm")
                nc.sync.dma_start(
                    h_pm[:], bout[:].rearrange("a (b c) -> (a b) c", b=PF)
                )
                h_loc = h_loc_new

            # keep the dummy psum live so DCE can't drop the warm-up matmuls
            if warm_dummies:
                nc.vector.tensor_copy(u[:, 0:1], scr[0:1, 0:1])
                nc.sync.dma_start(dbg[:], u[:, 0:1])

            # ---- outputs: p_ slice ----
            nc.sync.dma_start(pout[:], h_loc[:])

            # ---- Phase D: M_out = 0.9*M + 0.1*(p+p_)(p-p_)^T (column shard) ----
            a_pm = sm.tile([128, F], f32, tag="apm")
            nc.vector.tensor_add(a_pm[:], p_pm[:], h_pm[:])
            # bdiff = p_loc - h_loc, staged through the (now dead) v tile
            nc.sync.dma_start(v[:], ploc[:])
            nc.vector.tensor_sub(v[:], v[:], h_loc[:])
            # broadcast 0.1*bdiff across 128 partitions via K=1 outer product
            oneY = sm.tile([1, 128], f32, tag="oneY")
            nc.vector.memset(oneY[:], YITA)
            B01 = sm.tile([128, LOCAL], f32, tag="B01")
            for (off, sz), tg in zip(NSPLITS, ["bb0", "bb1", "bb2"]):
                bb = ps.tile([128, sz], f32, tag=tg, name=tg)
                nc.tensor.matmul(
                    bb[:], oneY[:], v[:, off:off + sz], start=True, stop=True
                )
                nc.vector.tensor_copy(B01[:, off:off + sz], bb[:])
            for t in range(F):
                w = rot.tile([128, LOCAL], f32, tag="w", name="w")
                ot = stpool.tile([128, LOCAL], f32, tag="stage", name="ot")
                nc.vector.tensor_scalar_mul(w[:], B01[:], a_pm[:, t:t + 1])
                nc.vector.scalar_tensor_tensor(
                    ot[:], M16s[t][:], LAMDA, w[:], Alu.mult, Alu.add,
                )
                nc.sync.dma_start(mout[t * 128:(t + 1) * 128, :], ot[:])

    nc.compile()
    return nc


def _get_nc():
    if "nc" not in _BUILD_CACHE:
        import os
        kw = {}
        if os.environ.get("KERNEL_COL_TILE"):
            kw["col_tile"] = int(os.environ["KERNEL_COL_TILE"])
        if os.environ.get("KERNEL_WARM_DUMMIES"):
            kw["warm_dummies"] = int(os.environ["KERNEL_WARM_DUMMIES"])
        _BUILD_CACHE["nc"] = _build_nc(**kw)
    return _BUILD_CACHE["nc"]


def _col_idx(c):
    """Global column indices owned by core c, in local (p'-major) order."""
    J = np.arange(P, dtype=np.int64).reshape(F, 128)  # J[f, p] = f*128 + p
    return J[:, PF * c: PF * (c + 1)].T.reshape(-1)   # (p', f) -> f*128+16c+p'


def _pm(vec):
    """[P] vector -> partition-major [128, F]: out[p, f] = vec[f*128 + p]."""
    return np.ascontiguousarray(vec.reshape(F, 128).T)


def kernel(x, g, M, W1x, b1x, W2x, b2x, W1g, b1g, W2g, b2g, Wp, bp):
    from concourse.bass_utils import run_bass_kernel_spmd

    f32 = np.float32
    x = np.asarray(x, f32); g = np.asarray(g, f32); M = np.asarray(M, f32)

    # ---- host: tiny MLPs, p, query, h0 ----
    def mlp2(inp, W1, b1, W2, b2):
        h = inp @ np.asarray(W1, f32) + np.asarray(b1, f32)
        return np.maximum(h, 0.0) @ np.asarray(W2, f32) + np.asarray(b2, f32)

    x_ = mlp2(x, W1x, b1x, W2x, b2x)[0]   # [96]
    g_ = mlp2(g, W1g, b1g, W2g, b2g)[0]   # [96]
    p = np.outer(x_, g_).reshape(-1).astype(f32)       # [9216]
    query = np.tile(g_, DIM_X).astype(f32)             # [9216]
    h0 = np.clip(np.maximum(query, LEAKY_SLOPE * query), -1.0, 1.0).astype(f32)

    M0 = M.reshape(P, P)
    arr = M0.reshape(P, F, 128)
    h0_pm = _pm(h0)
    p_pm = _pm(p)

    in_maps = []
    for c in range(NCORES):
        shard = np.ascontiguousarray(
            arr[:, :, PF * c: PF * (c + 1)].transpose(0, 2, 1).reshape(P, LOCAL)
        )
        h0_loc = np.ascontiguousarray(h0_pm[PF * c: PF * (c + 1), :]).reshape(1, LOCAL)
        p_loc = np.ascontiguousarray(p_pm[PF * c: PF * (c + 1), :]).reshape(1, LOCAL)
        in_maps.append({
            "m_in": shard,
            "h0pm": h0_pm.astype(np.float16),
            "h0loc": h0_loc,
            "ppm": p_pm,
            "ploc": p_loc,
        })

    nc = _get_nc()
    res = run_bass_kernel_spmd(nc, in_maps, core_ids=list(range(NCORES)))
    results = res.results

    # ---- host: reassemble ----
    p_pm_full = np.empty((128, F), f32)
    R = np.empty((P, F, 128), f32)
    for c in range(NCORES):
        p_pm_full[PF * c: PF * (c + 1), :] = results[c]["pout"].reshape(PF, F)
        R[:, :, PF * c: PF * (c + 1)] = (
            results[c]["mout"].reshape(P, PF, F).transpose(0, 2, 1)
        )
    p_ = np.ascontiguousarray(p_pm_full.T).reshape(1, P)
    M_out = R.reshape(1, P, P)

    x_out = p_.reshape(DIM_X, DIM_G).sum(axis=1)
    x_inf = (x_out @ np.asarray(Wp, f32) + np.asarray(bp, f32)).astype(f32)

    return x_inf, p_, M_out


# revision 19
# speedup vs baseline: 2.2699x; 1.0735x over previous
"""Trainium2 Bass kernel for nn_MemoryGame (scatter_memory).

Strategy (8 NeuronCores, tensor-parallel over M's columns):
  - Tiny MLPs / outer products / readout run on host (microseconds of work).
  - The heavy part (50-iteration Hopfield loop over M [9216,9216] plus the
    rank-1 Hebbian update of M) runs on 8 cores.
  - M's columns are sharded interleaved: core c owns columns j with
    (j % 128)//16 == c, stored locally in (p', f) order where the global
    column is j = f*128 + 16c + p'.  With this order, the per-iteration
    AllGather of the 8 local h-slices concatenates into exactly the
    partition-major [128, 72] layout the TensorEngine needs for the next
    matvec -- every DMA stays contiguous.
  - M is cast to fp16 and kept resident in SBUF (21.2MB/core): HBM traffic is
    one 42.5MB read (load) and one 42.5MB write (M_out) per core, total.
  - h state stays f32 (it decays to ~1e-5, below fp16-normal range); the
    fp16 rounding only enters the matvec input, where its effect on the
    update factor (kappa + h@M) is suppressed by kappa.
  - The AllGather transports the fp16 matvec copy of h; dummy matmuls keep
    the PE's HAM clock-gate warm across the gather tail.
"""

import numpy as np

P = 9216
DIM_X = 96
DIM_G = 96
NUM_CLASS = 1000
N_ITER = 50
KAPPA, LAMDA, YITA = 0.8, 0.9, 0.1
LEAKY_SLOPE = 0.01

NCORES = 8
LOCAL = P // NCORES          # 1152 columns per core
F = P // 128                 # 72 k-chunks / free size of pm layout
PF = 128 // NCORES           # 16 partitions per core in the interleave
NSPLITS = [(0, 512), (512, 512), (1024, 128)]  # LOCAL split into <=512 psum slices

_BUILD_CACHE = {}


def _build_nc(n_iter=N_ITER, use_cc=True, warm_dummies=0, col_tile=0):
    import concourse.bacc as bacc
    import concourse.mybir as mybir
    import concourse.tile as tile

    f32 = mybir.dt.float32
    f16 = mybir.dt.float16
    Alu = mybir.AluOpType

    nc = bacc.Bacc("TRN2", target_bir_lowering=False, num_devices=NCORES)

    m_in = nc.dram_tensor("m_in", [P, LOCAL], f32, kind="ExternalInput")
    h0pm = nc.dram_tensor("h0pm", [128, F], f16, kind="ExternalInput")
    h0loc = nc.dram_tensor("h0loc", [1, LOCAL], f32, kind="ExternalInput")
    ppm = nc.dram_tensor("ppm", [128, F], f32, kind="ExternalInput")
    ploc = nc.dram_tensor("ploc", [1, LOCAL], f32, kind="ExternalInput")
    pout = nc.dram_tensor("pout", [1, LOCAL], f32, kind="ExternalOutput")
    mout = nc.dram_tensor("mout", [P, LOCAL], f32, kind="ExternalOutput")

    rg = [list(range(NCORES))]

    with tile.TileContext(nc) as tc:
        with (
            tc.tile_pool(name="mpool", bufs=1) as mpool,
            tc.tile_pool(name="stage", bufs=2) as stpool,
            tc.tile_pool(name="small", bufs=1) as sm,
            tc.tile_pool(name="rot", bufs=2) as rot,
            tc.tile_pool(name="psum", bufs=1, space="PSUM") as ps,
            tc.tile_pool(name="dram", bufs=max(n_iter, 2), space="DRAM") as dr,
        ):
            # ---- Phase A: load M shard, cast f32 -> fp16 into SBUF ----
            M16s = []
            for t in range(F):
                mt = mpool.tile([128, LOCAL], f16, tag=f"m{t}", name=f"m{t}")
                M16s.append(mt)
                stg = stpool.tile([128, LOCAL], f32, tag="stage", name="stg")
                eng = nc.sync if t % 2 == 0 else nc.scalar
                eng.dma_start(stg[:], m_in[t * 128:(t + 1) * 128, :])
                nc.vector.tensor_copy(mt[:], stg[:])

            # ---- persistent small tiles ----
            p_pm = sm.tile([128, F], f32, tag="p_pm")
            nc.sync.dma_start(p_pm[:], ppm[:])
            u = sm.tile([1, LOCAL], f32, tag="u")
            v = sm.tile([1, LOCAL], f32, tag="v")
            h16d = sm.tile([128, 1], f16, tag="h16d")  # constant dummy weights
            nc.vector.memset(h16d[:], 0.0)

            h_pm = rot.tile([128, F], f16, tag="hpm", name="hpm0")
            nc.sync.dma_start(h_pm[:], h0pm[:])
            h_loc = rot.tile([1, LOCAL], f32, tag="hloc", name="hloc0")
            nc.sync.dma_start(h_loc[:], h0loc[:])

            if col_tile:
                acc = ps.tile([128, 512], f32, tag="acc", name="acc")
                csz = LOCAL // col_tile
            else:
                acc = ps.tile([1, 1536], f32, tag="acc", name="acc")
            scr = ps.tile([1, 512], f32, tag="scr", name="scr")
            dbg = dr.tile([1, 1], f32, tag="dbg", bufs=1, name="dbg")

            def dummy_mm(dep_ap=None):
                # keep the PE HAM-warm; reads a constant weight + M chunk 0
                lhsT = dep_ap if dep_ap is not None else h16d[:]
                nc.tensor.matmul(scr[:], lhsT, M16s[0][:, 0:512],
                                 start=True, stop=True, skip_group_check=True)

            # ---- Phase B: Hopfield attractor loop ----
            for it in range(n_iter):
                if col_tile:
                    for t in range(F):
                        lhsT = h_pm[:, t:t + 1]
                        for g in range(col_tile):
                            nc.tensor.matmul(
                                acc[32 * g:32 * g + 1, 0:csz],
                                lhsT,
                                M16s[t][:, g * csz:(g + 1) * csz],
                                start=(t == 0), stop=(t == F - 1),
                                tile_position=(0, 32 * g),
                            )
                else:
                    for t in range(F):
                        lhsT = h_pm[:, t:t + 1]
                        for off, sz in NSPLITS:
                            nc.tensor.matmul(
                                acc[:, off:off + sz],
                                lhsT,
                                M16s[t][:, off:off + sz],
                                start=(t == 0), stop=(t == F - 1),
                            )
                # u = (hM + kappa) * h
                if col_tile:
                    for g in range(col_tile):
                        nc.vector.scalar_tensor_tensor(
                            u[:, g * csz:(g + 1) * csz],
                            acc[32 * g:32 * g + 1, 0:csz], KAPPA,
                            h_loc[:, g * csz:(g + 1) * csz],
                            Alu.add, Alu.mult,
                        )
                else:
                    nc.vector.scalar_tensor_tensor(
                        u[:], acc[:, 0:LOCAL], KAPPA, h_loc[:],
                        Alu.add, Alu.mult,
                    )
                if warm_dummies:
                    dummy_mm()
                # leaky relu: v = max(u*0.01, u)
                nc.vector.scalar_tensor_tensor(
                    v[:], u[:], LEAKY_SLOPE, u[:], Alu.mult, Alu.max
                )
                if warm_dummies:
                    dummy_mm()
                # clamp to [-1, 1]
                h_loc_new = rot.tile([1, LOCAL], f32, tag="hloc", name="hloc")
                nc.vector.tensor_scalar(
                    h_loc_new[:], v[:], 1.0, -1.0, Alu.min, Alu.max
                )
                # fp16 transport copy
                h16loc = rot.tile([1, LOCAL], f16, tag="h16loc", name="h16loc",
                                  bufs=1)
                nc.vector.tensor_copy(h16loc[:], h_loc_new[:])
                if warm_dummies:
                    for _ in range(warm_dummies):
                        dummy_mm()

                # all-gather the 8 local slices into pm order (fp16)
                bin_ = dr.tile([1, LOCAL], f16, tag="bin", name="bin")
                bout = dr.tile([NCORES, LOCAL], f16, tag="bout", name="bout")
                nc.sync.dma_start(bin_[:], h16loc[:])
                if use_cc:
                    nc.gpsimd.collective_compute(
                        "AllGather", Alu.bypass, replica_groups=rg,
                        ins=[bin_[:].opt()], outs=[bout[:].opt()],
                    )
                else:  # debug: fake gather (wrong data, same dataflow)
                    for r in range(NCORES):
                        nc.sync.dma_start(bout[r:r + 1, :], bin_[:])
                h_pm = rot.tile([128, F], f16, tag="hpm", name="hpm")
                nc.sync.dma_start(
                    h_pm[:], bout[:].rearrange("a (b c) -> (a b) c", b=PF)
                )
                h_loc = h_loc_new

            # keep the dummy psum live so DCE can't drop the warm-up matmuls
            if warm_dummies:
                nc.vector.tensor_copy(u[:, 0:1], scr[0:1, 0:1])
                nc.sync.dma_start(dbg[:], u[:, 0:1])

            # ---- outputs: p_ slice ----
            nc.sync.dma_start(pout[:], h_loc[:])

            # ---- Phase D: M_out = 0.9*M + 0.1*(p+p_)(p-p_)^T (column shard) ----
            a_pm = sm.tile([128, F], f32, tag="apm")
            nc.vector.tensor_add(a_pm[:], p_pm[:], h_pm[:])
            # bdiff = p_loc - h_loc, staged through the (now dead) v tile
            nc.sync.dma_start(v[:], ploc[:])
            nc.vector.tensor_sub(v[:], v[:], h_loc[:])
            # broadcast 0.1*bdiff across 128 partitions via K=1 outer product
            oneY = sm.tile([1, 128], f32, tag="oneY")
            nc.vector.memset(oneY[:], YITA)
            B01 = sm.tile([128, LOCAL], f32, tag="B01")
            for (off, sz), tg in zip(NSPLITS, ["bb0", "bb1", "bb2"]):
                bb = ps.tile([128, sz], f32, tag=tg, name=tg)
                nc.tensor.matmul(
                    bb[:], oneY[:], v[:, off:off + sz], start=True, stop=True
                )
                nc.vector.tensor_copy(B01[:, off:off + sz], bb[:])
            Act = mybir.ActivationFunctionType
            for t in range(F):
                w = rot.tile([128, LOCAL], f32, tag="w", name="w")
                ot = stpool.tile([128, LOCAL], f32, tag="stage", name="ot")
                if t % 2 == 0:
                    # ScalarE: w = B01 * a_t (per-partition scale)
                    nc.scalar.activation(w[:], B01[:], Act.Copy,
                                         scale=a_pm[:, t:t + 1])
                else:
                    nc.vector.tensor_scalar_mul(w[:], B01[:], a_pm[:, t:t + 1])
                nc.vector.scalar_tensor_tensor(
                    ot[:], M16s[t][:], LAMDA, w[:], Alu.mult, Alu.add,
                )
                nc.sync.dma_start(mout[t * 128:(t + 1) * 128, :], ot[:])

    nc.compile()
    return nc


def _get_nc():
    if "nc" not in _BUILD_CACHE:
        import os
        kw = {}
        if os.environ.get("KERNEL_COL_TILE"):
            kw["col_tile"] = int(os.environ["KERNEL_COL_TILE"])
        if os.environ.get("KERNEL_WARM_DUMMIES"):
            kw["warm_dummies"] = int(os.environ["KERNEL_WARM_DUMMIES"])
        _BUILD_CACHE["nc"] = _build_nc(**kw)
    return _BUILD_CACHE["nc"]


def _col_idx(c):
    """Global column indices owned by core c, in local (p'-major) order."""
    J = np.arange(P, dtype=np.int64).reshape(F, 128)  # J[f, p] = f*128 + p
    return J[:, PF * c: PF * (c + 1)].T.reshape(-1)   # (p', f) -> f*128+16c+p'


def _pm(vec):
    """[P] vector -> partition-major [128, F]: out[p, f] = vec[f*128 + p]."""
    return np.ascontiguousarray(vec.reshape(F, 128).T)


def kernel(x, g, M, W1x, b1x, W2x, b2x, W1g, b1g, W2g, b2g, Wp, bp):
    from concourse.bass_utils import run_bass_kernel_spmd

    f32 = np.float32
    x = np.asarray(x, f32); g = np.asarray(g, f32); M = np.asarray(M, f32)

    # ---- host: tiny MLPs, p, query, h0 ----
    def mlp2(inp, W1, b1, W2, b2):
        h = inp @ np.asarray(W1, f32) + np.asarray(b1, f32)
        return np.maximum(h, 0.0) @ np.asarray(W2, f32) + np.asarray(b2, f32)

    x_ = mlp2(x, W1x, b1x, W2x, b2x)[0]   # [96]
    g_ = mlp2(g, W1g, b1g, W2g, b2g)[0]   # [96]
    p = np.outer(x_, g_).reshape(-1).astype(f32)       # [9216]
    query = np.tile(g_, DIM_X).astype(f32)             # [9216]
    h0 = np.clip(np.maximum(query, LEAKY_SLOPE * query), -1.0, 1.0).astype(f32)

    M0 = M.reshape(P, P)
    arr = M0.reshape(P, F, 128)
    h0_pm = _pm(h0)
    p_pm = _pm(p)

    in_maps = []
    for c in range(NCORES):
        shard = np.ascontiguousarray(
            arr[:, :, PF * c: PF * (c + 1)].transpose(0, 2, 1).reshape(P, LOCAL)
        )
        h0_loc = np.ascontiguousarray(h0_pm[PF * c: PF * (c + 1), :]).reshape(1, LOCAL)
        p_loc = np.ascontiguousarray(p_pm[PF * c: PF * (c + 1), :]).reshape(1, LOCAL)
        in_maps.append({
            "m_in": shard,
            "h0pm": h0_pm.astype(np.float16),
            "h0loc": h0_loc,
            "ppm": p_pm,
            "ploc": p_loc,
        })

    nc = _get_nc()
    res = run_bass_kernel_spmd(nc, in_maps, core_ids=list(range(NCORES)))
    results = res.results

    # ---- host: reassemble ----
    p_pm_full = np.empty((128, F), f32)
    R = np.empty((P, F, 128), f32)
    for c in range(NCORES):
        p_pm_full[PF * c: PF * (c + 1), :] = results[c]["pout"].reshape(PF, F)
        R[:, :, PF * c: PF * (c + 1)] = (
            results[c]["mout"].reshape(P, PF, F).transpose(0, 2, 1)
        )
    p_ = np.ascontiguousarray(p_pm_full.T).reshape(1, P)
    M_out = R.reshape(1, P, P)

    x_out = p_.reshape(DIM_X, DIM_G).sum(axis=1)
    x_inf = (x_out @ np.asarray(Wp, f32) + np.asarray(bp, f32)).astype(f32)

    return x_inf, p_, M_out
